# revision 21
# baseline (speedup 1.0000x reference)
"""DCGRU cell (nn_DCGRUCell) Trainium2 Bass kernel, 8 NeuronCores.

Sharding: node dimension N=4096 split 8 ways (512 rows/core); supports are
fed host-transposed (T = A^T) so tensor-engine matmuls need no on-device
transposes of A. Hop-1 diffusion products are computed node-major
[node, (batch, feat)] and AllGathered across cores; hop-2 products are
computed directly in transposed (feature-major) form since they only feed
the dense W stage. All matmuls run fp32 (float32r streaming); PSUM fp32.

kernel(**inputs) takes the FULL inputs from reference.setup_inputs() and
returns the FULL [16, 4096, 64] float32 output.
"""
import os
import numpy as np

import concourse.bass as bass
import concourse.mybir as mybir
import concourse.tile as tile
from concourse import bacc
from concourse.bass_utils import run_bass_kernel_spmd

F32 = mybir.dt.float32
F32R = mybir.dt.float32r
AF = mybir.ActivationFunctionType

NCORES = 8
B, N, H, DIN = 16, 4096, 64, 2
C = DIN + H                 # 66 features per batch into each GCN
BC = B * C                  # 1056
NOWN = N // NCORES          # 512 rows per core
NT = NOWN // 128            # 4 n-tiles per core
MT = N // 128               # 32 m-tiles (contraction)
MAIN = 1024                 # bc columns in the node-major main sweep
RAG = BC - MAIN             # 32 ragged columns
JT = BC // 128              # 8 full 128-col feature tiles (+1 ragged)
MAIN_ELEMS = NOWN * MAIN
RAG_ELEMS = NT * 128 * RAG
SHARD = MAIN_ELEMS + RAG_ELEMS
GROUP = [list(range(NCORES))]

_NC_CACHE = {}


def build_nc():
    nc = bacc.Bacc("TRN2", target_bir_lowering=False, debug=False,
                   num_devices=NCORES)

    d = {}
    d["Ts"] = nc.dram_tensor("Ts", [2, N, NOWN], F32R, kind="ExternalInput")
    d["xs_main"] = nc.dram_tensor("xs_main", [N, MAIN], F32R,
                                  kind="ExternalInput")
    d["xs_rag"] = nc.dram_tensor("xs_rag", [MT, 128, RAG], F32R,
                                 kind="ExternalInput")
    d["xsT_orig"] = nc.dram_tensor("xsT_orig", [BC, NOWN], F32R,
                                   kind="ExternalInput")
    d["xsT_own"] = nc.dram_tensor("xsT_own", [BC, NOWN], F32R,
                                  kind="ExternalInput")
    d["Wg"] = nc.dram_tensor("Wg", [5 * C, 2 * H], F32R, kind="ExternalInput")
    d["bg"] = nc.dram_tensor("bg", [2 * H, 1], F32, kind="ExternalInput")
    d["Wu"] = nc.dram_tensor("Wu", [5 * C, H], F32R, kind="ExternalInput")
    d["bu"] = nc.dram_tensor("bu", [H, 1], F32, kind="ExternalInput")
    d["negI"] = nc.dram_tensor("negI", [128, 128], F32R, kind="ExternalInput")
    d["outT"] = nc.dram_tensor("outT", [B, H, NOWN], F32,
                               kind="ExternalOutput")

    with tile.TileContext(nc) as tc:
        _emit(nc, tc, d)
    nc.compile()
    return nc


def _emit(nc, tc, d):
    import contextlib
    stack = contextlib.ExitStack()
    with stack:
        const = stack.enter_context(tc.tile_pool(name="const", bufs=1))
        sb_ex = stack.enter_context(tc.tile_pool(name="ex", bufs=1))
        sb_mov = stack.enter_context(tc.tile_pool(name="mov", bufs=1))
        sb_sm = stack.enter_context(tc.tile_pool(name="small", bufs=1))
        dram = stack.enter_context(
            tc.tile_pool(name="dram", bufs=1, space="DRAM"))
        psum = stack.enter_context(
            tc.tile_pool(name="psum", bufs=1, space="PSUM"))

        # ---- constants / resident tensors ----
        # supports loaded in interleaved 4-m-tile chunks so the first
        # matmuls only wait for the first small chunk
        CH = 4
        NCH = MT // CH
        Tch = {}
        for s in range(2):
            for k in range(NCH):
                Tch[(s, k)] = const.tile([128, CH, 512], F32R,
                                         name=f"T{s}_{k}")
        for k in range(NCH):
            for s in range(2):
                ts = d["Ts"].ap()[s].rearrange("(t p) n -> p t n", p=128)
                nc.sync.dma_start(Tch[(s, k)][:],
                                  ts[:, k * CH:(k + 1) * CH, :])

        def T_tile(s, m):
            return Tch[(s, m // CH)][:, m % CH, :]

        ident = const.tile([128, 128], F32)
        nc.gpsimd.memset(ident[:], 0.0)
        nc.gpsimd.affine_select(
            out=ident[:], in_=ident[:],
            compare_op=mybir.AluOpType.not_equal, fill=1.0, base=0,
            pattern=[[-1, 128]], channel_multiplier=1)
        nident = const.tile([128, 128], F32R)  # -0.5*I (for 2*(Ax - X/2))
        nc.sync.dma_start(nident[:], d["negI"].ap())

        wg_t = const.tile([C, 5, 2 * H], F32R)
        wu_t = const.tile([C, 5, H], F32R)
        for j in range(5):
            nc.sync.dma_start(wg_t[:, j, :],
                              d["Wg"].ap()[j * C:(j + 1) * C, :])
            nc.sync.dma_start(wu_t[:, j, :],
                              d["Wu"].ap()[j * C:(j + 1) * C, :])
        bg_t = const.tile([2 * H, 1], F32)
        nc.sync.dma_start(bg_t[:], d["bg"].ap())
        bu_t = const.tile([H, 1], F32)
        nc.sync.dma_start(bu_t[:], d["bu"].ap())

        # ---- DRAM staging ----
        # AG slots: 0,1 = y1 of gcn1; 2 = cand; 3,4 = y1 of gcn2
        ag_in = [dram.tile([SHARD], F32R, name=f"agin{i}") for i in range(5)]
        ag_out = [dram.tile([NCORES * SHARD], F32R, name=f"agout{i}",
                            addr_space="Shared") for i in range(5)]
        yt = [dram.tile([BC, NOWN], F32R, name=f"yt{i}") for i in range(4)]
        yt2 = [dram.tile([BC, NOWN], F32R, name=f"yt2_{i}") for i in range(4)]
        candT_dram = dram.tile([BC, NOWN], F32R)
        rt_dram = dram.tile([B, H, NOWN], F32)

        def xs_main_half(m, h):
            return d["xs_main"].ap()[m * 128:(m + 1) * 128,
                                     h * 512:(h + 1) * 512]

        def xs_rag_tile(m):
            return d["xs_rag"].ap()[m]

        def ag_main_half(i, m, h):
            a = ag_out[i].opt()
            off = (m // NT) * SHARD + (m % NT) * 128 * MAIN
            v = a[off:off + 128 * MAIN].rearrange("(p f) -> p f", f=MAIN)
            return v[:, h * 512:(h + 1) * 512]

        def ag_rag_tile(i, m):
            a = ag_out[i].opt()
            off = (m // NT) * SHARD + MAIN_ELEMS + (m % NT) * 128 * RAG
            return a[off:off + 128 * RAG].rearrange("(p f) -> p f", f=RAG)

        def ag_col_chunk(i, q, j):
            """[128, NT, 128] moving chunk: rank q's own rows, feature
            columns 128j..128(j+1)."""
            a = ag_out[i].opt()
            blk = a[q * SHARD:q * SHARD + MAIN_ELEMS].rearrange(
                "(t p f) -> p t f", p=128, f=MAIN)
            return blk[:, :, j * 128:(j + 1) * 128]

        def agin_own_main(i, t):
            a = ag_in[i].opt()
            return a[t * 128 * MAIN:(t + 1) * 128 * MAIN].rearrange(
                "(p f) -> p f", f=MAIN)

        def agin_own_main_half(i, t, h):
            return agin_own_main(i, t)[:, h * 512:(h + 1) * 512]

        def agin_own_rag(i, t):
            a = ag_in[i].opt()
            off = MAIN_ELEMS + t * 128 * RAG
            return a[off:off + 128 * RAG].rearrange("(p f) -> p f", f=RAG)

        # ============ hop-1 products: node-major + AllGather ============
        # Both supports share one pass over the moving operand.
        def emit_hop1_pair(pid, mov_main, mov_rag, agin_idx, yt_dst):
            """Y1_s[own rows, :] = A_s @ M for s in (0, 1)."""
            # ragged columns, transposed: psum[c(32), n(512)] per support
            ps_t = [psum.tile([RAG, NOWN], F32, name=f"pst{pid}{s}",
                              tag="acc", bufs=8) for s in range(2)]
            for m in range(MT):
                mvr = sb_mov.tile([128, RAG], F32R, name=f"mvr{pid}_{m}",
                                  tag="movr", bufs=8)
                nc.sync.dma_start(mvr[:], mov_rag(m))
                for s in range(2):
                    nc.tensor.matmul(ps_t[s][:], mvr[:], T_tile(s, m),
                                     start=(m == 0), stop=(m == MT - 1))
            for s in range(2):
                rag_ex = sb_sm.tile([RAG, NOWN], F32, name=f"rgex{pid}{s}",
                                    tag="ragex", bufs=1)
                nc.vector.tensor_copy(rag_ex[:], ps_t[s][:])
                nc.sync.dma_start(yt_dst[s].opt()[MAIN:BC, :].bitcast(F32),
                                  rag_ex[:])
                for t in range(NT):
                    tp = psum.tile([128, RAG], F32, name=f"rtp{pid}{s}",
                                   tag="acc", bufs=8)
                    nc.tensor.transpose(
                        tp[:], rag_ex[:, t * 128:(t + 1) * 128],
                        ident[0:RAG, 0:RAG])
                    rnm = sb_sm.tile([128, RAG], F32, name=f"rnm{pid}{s}",
                                     tag="rnm", bufs=2)
                    nc.vector.tensor_copy(rnm[:], tp[:])
                    nc.sync.dma_start(
                        agin_own_rag(agin_idx[s], t).bitcast(F32), rnm[:])

            # main columns in two 512-wide sweeps; 2 supports x 4 n-tiles
            # of accumulators fill all 8 PSUM banks per sweep
            for hh in range(2):
                ps_m = {}
                for s in range(2):
                    for n in range(NT):
                        ps_m[(s, n)] = psum.tile(
                            [128, 512], F32, name=f"psm{pid}_{hh}{s}{n}",
                            tag="acc", bufs=8)
                for m in range(MT):
                    mv = sb_mov.tile([128, 512], F32R,
                                     name=f"mv{pid}_{hh}_{m}", tag="mov",
                                     bufs=5)
                    nc.sync.dma_start(mv[:], mov_main(m, hh))
                    for s in range(2):
                        for n in range(NT):
                            nc.tensor.matmul(
                                ps_m[(s, n)][:],
                                T_tile(s, m)[:, n * 128:(n + 1) * 128],
                                mv[:], start=(m == 0), stop=(m == MT - 1))
                for s in range(2):
                    exhs = []
                    for n in range(NT):
                        exh = sb_ex.tile([128, 512], F32,
                                         name=f"ex{pid}{hh}{s}{n}",
                                         tag="ex", bufs=5)
                        nc.vector.tensor_copy(exh[:], ps_m[(s, n)][:])
                        nc.sync.dma_start(
                            agin_own_main_half(agin_idx[s], n, hh)
                            .bitcast(F32), exh[:])
                        exhs.append(exh)
                    # feature-major staging: per bc row-block j, transpose
                    # the 4 n-chunks and write one contiguous row-block
                    for j in range(4):
                        st4 = sb_sm.tile([128, NOWN], F32,
                                         name=f"st4{pid}", tag="st", bufs=2)
                        for n in range(NT):
                            tp = psum.tile([128, 128], F32,
                                           name=f"tp{pid}", tag="acc",
                                           bufs=8)
                            nc.tensor.transpose(
                                tp[:], exhs[n][:, j * 128:(j + 1) * 128],
                                ident[:])
                            nc.vector.tensor_copy(
                                st4[:, n * 128:(n + 1) * 128], tp[:])
                        jj = hh * 4 + j
                        nc.sync.dma_start(
                            yt_dst[s].opt()[jj * 128:(jj + 1) * 128, :]
                            .bitcast(F32), st4[:])
            nc.gpsimd.collective_compute(
                "AllGather", mybir.AluOpType.bypass, replica_groups=GROUP,
                ins=[ag_in[agin_idx[0]].opt()],
                outs=[ag_out[agin_idx[0]].opt()])
            nc.gpsimd.collective_compute(
                "AllGather", mybir.AluOpType.bypass, replica_groups=GROUP,
                ins=[ag_in[agin_idx[1]].opt()],
                outs=[ag_out[agin_idx[1]].opt()])

        # ======= hop-2 product: transposed form (feature-major out) =======
        def emit_hop2(pid, s, ag_idx, ownT_rows, yt_dst):
            """Y2^T[bc, own n] = 2*(A_s @ Y1)^T[bc, n] - X^T[bc, n].

            Moving operand = gathered Y1 (ag_out[ag_idx]) loaded as full
            m-rows; its 128-col slices act as lhsT for 8 concurrent
            feature-tile accumulators. ownT_rows(j, w) gives X^T rows for
            the -X term."""
            # ragged feature tile (j = JT), its own accumulation
            ps_r = psum.tile([RAG, NOWN], F32, name=f"ph2r{pid}", tag="acc",
                             bufs=8)
            for m in range(MT):
                mvr = sb_mov.tile([128, RAG], F32R, name=f"mvr{pid}_{m}",
                                  tag="movr", bufs=8)
                nc.sync.dma_start(mvr[:], ag_rag_tile(ag_idx, m))
                nc.tensor.matmul(ps_r[:], mvr[:], T_tile(s, m),
                                 start=(m == 0), stop=False)
            xrt = sb_mov.tile([RAG, NOWN], F32R, name=f"xrt{pid}r",
                              tag="xrt", bufs=2)
            nc.sync.dma_start(xrt[:], ownT_rows(JT, RAG))
            nc.tensor.matmul(ps_r[:], nident[0:RAG, 0:RAG], xrt[:],
                             start=False, stop=True)
            exr = sb_ex.tile([RAG, NOWN], F32, name=f"h2exr{pid}",
                             tag="ex", bufs=5)
            nc.scalar.mul(exr[:], ps_r[:], 2.0)
            nc.sync.dma_start(
                yt_dst.opt()[MAIN:BC, :].bitcast(F32), exr[:])

            # 8 full feature tiles, m-outer (row loads are contiguous)
            ps = [psum.tile([128, NOWN], F32, name=f"ph2{pid}_{j}",
                            tag="acc", bufs=8) for j in range(JT)]
            for m in range(MT):
                mrow = sb_mov.tile([128, MAIN], F32R, name=f"mr{pid}_{m}",
                                   tag="mov", bufs=5)
                for h in range(2):
                    nc.sync.dma_start(mrow[:, h * 512:(h + 1) * 512],
                                      ag_main_half(ag_idx, m, h))
                for j in range(JT):
                    nc.tensor.matmul(
                        ps[j][:], mrow[:, j * 128:(j + 1) * 128],
                        T_tile(s, m), start=(m == 0), stop=False)
            for j in range(JT):
                xrt = sb_mov.tile([128, NOWN], F32R, name=f"xrt{pid}_{j}",
                                  tag="xrt", bufs=2)
                nc.sync.dma_start(xrt[:], ownT_rows(j, 128))
                nc.tensor.matmul(ps[j][:], nident[:], xrt[:],
                                 start=False, stop=True)
                exh = sb_ex.tile([128, NOWN], F32, name=f"h2ex{pid}_{j}",
                                 tag="ex", bufs=5)
                nc.scalar.mul(exh[:], ps[j][:], 2.0)
                nc.sync.dma_start(
                    yt_dst.opt()[j * 128:(j + 1) * 128, :].bitcast(F32),
                    exh[:])

        # ======================= GCN 1 (gate) =======================
        emit_hop1_pair("g1h1", xs_main_half, xs_rag_tile, (0, 1),
                       (yt[0], yt[2]))

        def xsT_orig_rows(j, w):
            return d["xsT_orig"].ap()[j * 128:j * 128 + w, :]

        emit_hop2("g1s0h2", 0, 0, xsT_orig_rows, yt[1])
        emit_hop2("g1s1h2", 1, 1, xsT_orig_rows, yt[3])

        # gate W-stage + candidate build
        for b in range(B):
            xsT_b = sb_sm.tile([C, NOWN], F32R, name="xsTb", tag="xsTb",
                               bufs=2)
            nc.sync.dma_start(xsT_b[:],
                              d["xsT_own"].ap()[b * C:(b + 1) * C, :])
            blocks = [xsT_b]
            for j in range(4):
                bt = sb_sm.tile([C, NOWN], F32R, name=f"blk{j}",
                                tag=f"blk{j}", bufs=1)
                nc.sync.dma_start(bt[:], yt[j].opt()[b * C:(b + 1) * C, :])
                blocks.append(bt)
            zr_ps = psum.tile([2 * H, NOWN], F32, name="zrps", tag="acc", bufs=8)
            for j in range(5):
                nc.tensor.matmul(zr_ps[:], wg_t[:, j, :], blocks[j][:],
                                 start=(j == 0), stop=(j == 4))
            zr = sb_sm.tile([2 * H, NOWN], F32, name="zr", tag="zr", bufs=2)
            nc.scalar.activation(zr[:], zr_ps[:], AF.Sigmoid, bias=bg_t[:])
            nc.sync.dma_start(rt_dram.opt()[b], zr[H:2 * H, :])
            # candT_b rows are [z*state(64); x(2)] (host permutes W rows)
            cT = sb_sm.tile([C, NOWN], F32, name="cT", tag="cT", bufs=1)
            nc.vector.tensor_mul(cT[0:H, :], zr[0:H, :],
                                 xsT_b[0:H, :].bitcast(F32))
            nc.vector.tensor_copy(cT[H:C, :], xsT_b[H:C, :].bitcast(F32))
            nc.sync.dma_start(
                candT_dram.opt()[b * C:(b + 1) * C, :].bitcast(F32), cT[:])
            # cand node-major -> ag_in[2]
            a_main = ag_in[2].opt()[0:MAIN_ELEMS].rearrange(
                "(p f) -> p f", f=MAIN).bitcast(F32)
            for t in range(NT):
                ps = psum.tile([128, C], F32, name="ctps", tag="acc", bufs=8)
                nc.tensor.transpose(ps[:], cT[:, t * 128:(t + 1) * 128],
                                    ident[0:C, 0:C])
                ct_nm = sb_sm.tile([128, C], F32, name="ctnm", tag="ctnm", bufs=1)
                nc.vector.tensor_copy(ct_nm[:], ps[:])
                lo, hi = b * C, (b + 1) * C
                if hi <= MAIN:
                    nc.sync.dma_start(
                        a_main[t * 128:(t + 1) * 128, lo:hi], ct_nm[:])
                else:
                    cut = MAIN - lo
                    nc.sync.dma_start(
                        a_main[t * 128:(t + 1) * 128, lo:MAIN],
                        ct_nm[:, 0:cut])
                    nc.sync.dma_start(agin_own_rag(2, t).bitcast(F32),
                                      ct_nm[:, cut:C])
        nc.gpsimd.collective_compute(
            "AllGather", mybir.AluOpType.bypass, replica_groups=GROUP,
            ins=[ag_in[2].opt()], outs=[ag_out[2].opt()])

        # ======================= GCN 2 (update) =======================
        emit_hop1_pair("g2h1",
                       lambda m, h: ag_main_half(2, m, h),
                       lambda m: ag_rag_tile(2, m), (3, 4),
                       (yt2[0], yt2[2]))

        def candT_rows(j, w):
            return candT_dram.opt()[j * 128:j * 128 + w, :]

        emit_hop2("g2s0h2", 0, 3, candT_rows, yt2[1])
        emit_hop2("g2s1h2", 1, 4, candT_rows, yt2[3])

        # update W-stage + final combine
        for b in range(B):
            cT_b = sb_sm.tile([C, NOWN], F32R, name="cTb", tag="xsTb",
                              bufs=2)
            nc.sync.dma_start(cT_b[:],
                              candT_dram.opt()[b * C:(b + 1) * C, :])
            blocks = [cT_b]
            for j in range(4):
                bt = sb_sm.tile([C, NOWN], F32R, name=f"ublk{j}",
                                tag=f"blk{j}", bufs=1)
                nc.sync.dma_start(bt[:], yt2[j].opt()[b * C:(b + 1) * C, :])
                blocks.append(bt)
            hc_ps = psum.tile([H, NOWN], F32, name="hcps", tag="acc", bufs=8)
            for j in range(5):
                nc.tensor.matmul(hc_ps[:], wu_t[:, j, :], blocks[j][:],
                                 start=(j == 0), stop=(j == 4))
            hc = sb_sm.tile([H, NOWN], F32, name="hc", tag="zr", bufs=2)
            nc.scalar.activation(hc[:], hc_ps[:], AF.Tanh, bias=bu_t[:])

            # out = hc + r * (state - hc);  stateT = xsT_own rows [0:H]
            xsT_b = sb_sm.tile([C, NOWN], F32, name="xsTb2", tag="cT",
                               bufs=1)
            nc.sync.dma_start(
                xsT_b[:],
                d["xsT_own"].ap()[b * C:(b + 1) * C, :].bitcast(F32))
            rT = sb_sm.tile([H, NOWN], F32, name="rT", tag="rT", bufs=1)
            nc.sync.dma_start(rT[:], rt_dram.opt()[b])
            tmp = sb_sm.tile([H, NOWN], F32, name="tmp", tag="tmp", bufs=2)
            nc.vector.tensor_sub(tmp[:], xsT_b[0:H, :], hc[:])
            nc.vector.tensor_mul(tmp[:], rT[:], tmp[:])
            ot = sb_sm.tile([H, NOWN], F32, name="ot", tag="ot", bufs=2)
            nc.vector.tensor_add(ot[:], hc[:], tmp[:])
            nc.sync.dma_start(d["outT"].ap()[b], ot[:])


def prepare_in_maps(x, state, support0, support1, W_gate, b_gate,
                    W_update, b_update):
    xs = np.concatenate([x, state], axis=-1)          # [B, N, C]
    xs_nm = np.ascontiguousarray(
        xs.transpose(1, 0, 2).reshape(N, BC)).astype(np.float32)
    # feature-major input for W / elementwise uses [state(64); x(2)] rows
    sx_nm = np.ascontiguousarray(
        np.concatenate([state, x], axis=-1)
        .transpose(1, 0, 2).reshape(N, BC)).astype(np.float32)
    perm = np.r_[DIN:C, 0:DIN]                 # [x, state] -> [state, x]
    Wg_dev = np.ascontiguousarray(W_gate, dtype=np.float32).copy()
    Wg_dev[0:C] = Wg_dev[0:C][perm]            # only the X-block reads xsT
    Wu_dev = np.ascontiguousarray(W_update, dtype=np.float32).copy()
    for j in range(5):                         # all of cand's blocks permute
        Wu_dev[j * C:(j + 1) * C] = Wu_dev[j * C:(j + 1) * C][perm]
    xs_main = np.ascontiguousarray(xs_nm[:, :MAIN])
    xs_rag = np.ascontiguousarray(xs_nm[:, MAIN:]).reshape(MT, 128, RAG)
    bg = np.ascontiguousarray(b_gate, dtype=np.float32).reshape(2 * H, 1)
    bu = np.ascontiguousarray(b_update, dtype=np.float32).reshape(H, 1)
    negI = (-0.5 * np.eye(128, dtype=np.float32))

    in_maps = []
    for r in range(NCORES):
        n0 = r * NOWN
        sl = xs_nm[n0:n0 + NOWN]
        in_maps.append({
            "Ts": np.ascontiguousarray(
                np.stack([support0[n0:n0 + NOWN, :].T,
                          support1[n0:n0 + NOWN, :].T])).astype(np.float32),
            "xs_main": xs_main,
            "xs_rag": xs_rag,
            "xsT_orig": np.ascontiguousarray(sl.T),
            "xsT_own": np.ascontiguousarray(sx_nm[n0:n0 + NOWN].T),
            "Wg": Wg_dev, "bg": bg, "Wu": Wu_dev, "bu": bu,
            "negI": negI,
        })
    return in_maps


def assemble_output(results):
    out = np.empty((B, N, H), dtype=np.float32)
    for r in range(NCORES):
        n0 = r * NOWN
        out[:, n0:n0 + NOWN, :] = results[r]["outT"].transpose(0, 2, 1)
    return out


def get_nc():
    if "nc" not in _NC_CACHE:
        _NC_CACHE["nc"] = build_nc()
    return _NC_CACHE["nc"]


def kernel(x, state, support0, support1, W_gate, b_gate, W_update, b_update):
    nc = get_nc()
    in_maps = prepare_in_maps(x, state, support0, support1,
                              W_gate, b_gate, W_update, b_update)
    prev = os.environ.get("BASS_NEVER_TRACE")
    os.environ["BASS_NEVER_TRACE"] = "1"
    try:
        res = run_bass_kernel_spmd(nc, in_maps, list(range(NCORES)),
                                   trace=False)
    finally:
        if prev is None:
            os.environ.pop("BASS_NEVER_TRACE", None)
        else:
            os.environ["BASS_NEVER_TRACE"] = prev
    return assemble_output(res.results)


# revision 23
# speedup vs baseline: 1.0464x; 1.0464x over previous
"""DCGRU cell (nn_DCGRUCell) Trainium2 Bass kernel, 8 NeuronCores.

Sharding: node dimension N=4096 split 8 ways (512 rows/core); supports are
fed host-transposed (T = A^T) so tensor-engine matmuls need no on-device
transposes of A. Hop-1 diffusion products are computed node-major
[node, (batch, feat)] and AllGathered across cores; hop-2 products are
computed directly in transposed (feature-major) form since they only feed
the dense W stage. All matmuls run fp32 (float32r streaming); PSUM fp32.

kernel(**inputs) takes the FULL inputs from reference.setup_inputs() and
returns the FULL [16, 4096, 64] float32 output.
"""
import os
import numpy as np

import concourse.bass as bass
import concourse.mybir as mybir
import concourse.tile as tile
from concourse import bacc
from concourse.bass_utils import run_bass_kernel_spmd

F32 = mybir.dt.float32
F32R = mybir.dt.float32r
AF = mybir.ActivationFunctionType

NCORES = 8
B, N, H, DIN = 16, 4096, 64, 2
C = DIN + H                 # 66 features per batch into each GCN
BC = B * C                  # 1056
NOWN = N // NCORES          # 512 rows per core
NT = NOWN // 128            # 4 n-tiles per core
MT = N // 128               # 32 m-tiles (contraction)
MAIN = 1024                 # bc columns in the node-major main sweep
RAG = BC - MAIN             # 32 ragged columns
JT = BC // 128              # 8 full 128-col feature tiles (+1 ragged)
MAIN_ELEMS = NOWN * MAIN
HALF_E = NOWN * 512
RAG_ELEMS = NT * 128 * RAG
SHARD = MAIN_ELEMS + RAG_ELEMS
GROUP = [list(range(NCORES))]

_NC_CACHE = {}


def build_nc():
    nc = bacc.Bacc("TRN2", target_bir_lowering=False, debug=False,
                   num_devices=NCORES)

    d = {}
    d["Ts"] = nc.dram_tensor("Ts", [2, N, NOWN], F32R, kind="ExternalInput")
    d["xs_main"] = nc.dram_tensor("xs_main", [N, MAIN], F32R,
                                  kind="ExternalInput")
    d["xs_rag"] = nc.dram_tensor("xs_rag", [MT, 128, RAG], F32R,
                                 kind="ExternalInput")
    d["xsT_orig"] = nc.dram_tensor("xsT_orig", [BC, NOWN], F32R,
                                   kind="ExternalInput")
    d["xsT_own"] = nc.dram_tensor("xsT_own", [BC, NOWN], F32R,
                                  kind="ExternalInput")
    d["Wg"] = nc.dram_tensor("Wg", [5 * C, 2 * H], F32R, kind="ExternalInput")
    d["bg"] = nc.dram_tensor("bg", [2 * H, 1], F32, kind="ExternalInput")
    d["Wu"] = nc.dram_tensor("Wu", [5 * C, H], F32R, kind="ExternalInput")
    d["bu"] = nc.dram_tensor("bu", [H, 1], F32, kind="ExternalInput")
    d["negI"] = nc.dram_tensor("negI", [128, 128], F32R, kind="ExternalInput")
    d["outT"] = nc.dram_tensor("outT", [B, H, NOWN], F32,
                               kind="ExternalOutput")

    with tile.TileContext(nc) as tc:
        _emit(nc, tc, d)
    nc.compile()
    return nc


def _emit(nc, tc, d):
    import contextlib
    stack = contextlib.ExitStack()
    with stack:
        const = stack.enter_context(tc.tile_pool(name="const", bufs=1))
        sb_ex = stack.enter_context(tc.tile_pool(name="ex", bufs=1))
        sb_mov = stack.enter_context(tc.tile_pool(name="mov", bufs=1))
        sb_sm = stack.enter_context(tc.tile_pool(name="small", bufs=1))
        dram = stack.enter_context(
            tc.tile_pool(name="dram", bufs=1, space="DRAM"))
        psum = stack.enter_context(
            tc.tile_pool(name="psum", bufs=1, space="PSUM"))

        # ---- constants / resident tensors ----
        # supports loaded in interleaved 4-m-tile chunks so the first
        # matmuls only wait for the first small chunk
        CH = 4
        NCH = MT // CH
        Tch = {}
        for s in range(2):
            for k in range(NCH):
                Tch[(s, k)] = const.tile([128, CH, 512], F32R,
                                         name=f"T{s}_{k}")
        for k in range(NCH):
            for s in range(2):
                ts = d["Ts"].ap()[s].rearrange("(t p) n -> p t n", p=128)
                nc.sync.dma_start(Tch[(s, k)][:],
                                  ts[:, k * CH:(k + 1) * CH, :])

        def T_tile(s, m):
            return Tch[(s, m // CH)][:, m % CH, :]

        ident = const.tile([128, 128], F32)
        nc.gpsimd.memset(ident[:], 0.0)
        nc.gpsimd.affine_select(
            out=ident[:], in_=ident[:],
            compare_op=mybir.AluOpType.not_equal, fill=1.0, base=0,
            pattern=[[-1, 128]], channel_multiplier=1)
        nident = const.tile([128, 128], F32R)  # -0.5*I (for 2*(Ax - X/2))
        nc.sync.dma_start(nident[:], d["negI"].ap())

        wg_t = const.tile([C, 5, 2 * H], F32R)
        wu_t = const.tile([C, 5, H], F32R)
        for j in range(5):
            nc.sync.dma_start(wg_t[:, j, :],
                              d["Wg"].ap()[j * C:(j + 1) * C, :])
            nc.sync.dma_start(wu_t[:, j, :],
                              d["Wu"].ap()[j * C:(j + 1) * C, :])
        bg_t = const.tile([2 * H, 1], F32)
        nc.sync.dma_start(bg_t[:], d["bg"].ap())
        bu_t = const.tile([H, 1], F32)
        nc.sync.dma_start(bu_t[:], d["bu"].ap())

        # ---- DRAM staging ----
        # AG slots: 0,1 = y1 of gcn1; 2 = cand; 3,4 = y1 of gcn2
        ag_in = [dram.tile([SHARD], F32R, name=f"agin{i}") for i in range(5)]
        ag_out = [dram.tile([NCORES * SHARD], F32R, name=f"agout{i}",
                            addr_space="Shared") for i in range(5)]
        BSH = HALF_E + RAG_ELEMS
        ag_inA = {i: dram.tile([HALF_E], F32R, name=f"aginA{i}")
                  for i in (0, 1, 3, 4)}
        ag_inB = {i: dram.tile([BSH], F32R, name=f"aginB{i}")
                  for i in (0, 1, 3, 4)}
        ag_outA = {i: dram.tile([NCORES * HALF_E], F32R, name=f"agoutA{i}",
                                addr_space="Shared") for i in (0, 1, 3, 4)}
        ag_outB = {i: dram.tile([NCORES * BSH], F32R, name=f"agoutB{i}",
                                addr_space="Shared") for i in (0, 1, 3, 4)}

        def aginH_own(i, t, h):
            a = (ag_inA if h == 0 else ag_inB)[i].opt()
            return a[t * 128 * 512:(t + 1) * 128 * 512].rearrange(
                "(p f) -> p f", f=512)

        def aginH_own_rag(i, t):
            a = ag_inB[i].opt()
            off = HALF_E + t * 128 * RAG
            return a[off:off + 128 * RAG].rearrange("(p f) -> p f", f=RAG)

        def agH_main(i, m, h):
            base = (ag_outA if h == 0 else ag_outB)[i].opt()
            sh = HALF_E if h == 0 else BSH
            off = (m // NT) * sh + (m % NT) * 128 * 512
            return base[off:off + 128 * 512].rearrange("(p f) -> p f", f=512)

        def agH_rag(i, m):
            a = ag_outB[i].opt()
            off = (m // NT) * BSH + HALF_E + (m % NT) * 128 * RAG
            return a[off:off + 128 * RAG].rearrange("(p f) -> p f", f=RAG)
        yt = [dram.tile([BC, NOWN], F32R, name=f"yt{i}") for i in range(4)]
        yt2 = [dram.tile([BC, NOWN], F32R, name=f"yt2_{i}") for i in range(4)]
        candT_dram = dram.tile([BC, NOWN], F32R)
        rt_dram = dram.tile([B, H, NOWN], F32)

        def xs_main_half(m, h):
            return d["xs_main"].ap()[m * 128:(m + 1) * 128,
                                     h * 512:(h + 1) * 512]

        def xs_rag_tile(m):
            return d["xs_rag"].ap()[m]

        def ag_main_half(i, m, h):
            a = ag_out[i].opt()
            off = (m // NT) * SHARD + (m % NT) * 128 * MAIN
            v = a[off:off + 128 * MAIN].rearrange("(p f) -> p f", f=MAIN)
            return v[:, h * 512:(h + 1) * 512]

        def ag_rag_tile(i, m):
            a = ag_out[i].opt()
            off = (m // NT) * SHARD + MAIN_ELEMS + (m % NT) * 128 * RAG
            return a[off:off + 128 * RAG].rearrange("(p f) -> p f", f=RAG)

        def ag_col_chunk(i, q, j):
            """[128, NT, 128] moving chunk: rank q's own rows, feature
            columns 128j..128(j+1)."""
            a = ag_out[i].opt()
            blk = a[q * SHARD:q * SHARD + MAIN_ELEMS].rearrange(
                "(t p f) -> p t f", p=128, f=MAIN)
            return blk[:, :, j * 128:(j + 1) * 128]

        def agin_own_main(i, t):
            a = ag_in[i].opt()
            return a[t * 128 * MAIN:(t + 1) * 128 * MAIN].rearrange(
                "(p f) -> p f", f=MAIN)

        def agin_own_main_half(i, t, h):
            return agin_own_main(i, t)[:, h * 512:(h + 1) * 512]

        def agin_own_rag(i, t):
            a = ag_in[i].opt()
            off = MAIN_ELEMS + t * 128 * RAG
            return a[off:off + 128 * RAG].rearrange("(p f) -> p f", f=RAG)

        # ============ hop-1 products: node-major + AllGather ============
        # Both supports share one pass over the moving operand.
        def emit_hop1_pair(pid, mov_main, mov_rag, agin_idx, yt_dst):
            """Y1_s[own rows, :] = A_s @ M for s in (0, 1)."""
            # ragged columns, transposed: psum[c(32), n(512)] per support
            ps_t = [psum.tile([RAG, NOWN], F32, name=f"pst{pid}{s}",
                              tag="acc", bufs=8) for s in range(2)]
            for m in range(MT):
                mvr = sb_mov.tile([128, RAG], F32R, name=f"mvr{pid}_{m}",
                                  tag="movr", bufs=8)
                nc.sync.dma_start(mvr[:], mov_rag(m))
                for s in range(2):
                    nc.tensor.matmul(ps_t[s][:], mvr[:], T_tile(s, m),
                                     start=(m == 0), stop=(m == MT - 1))
            for s in range(2):
                rag_ex = sb_sm.tile([RAG, NOWN], F32, name=f"rgex{pid}{s}",
                                    tag="ragex", bufs=1)
                nc.vector.tensor_copy(rag_ex[:], ps_t[s][:])
                nc.sync.dma_start(yt_dst[s].opt()[MAIN:BC, :].bitcast(F32),
                                  rag_ex[:])
                for t in range(NT):
                    tp = psum.tile([128, RAG], F32, name=f"rtp{pid}{s}",
                                   tag="acc", bufs=8)
                    nc.tensor.transpose(
                        tp[:], rag_ex[:, t * 128:(t + 1) * 128],
                        ident[0:RAG, 0:RAG])
                    rnm = sb_sm.tile([128, RAG], F32, name=f"rnm{pid}{s}",
                                     tag="rnm", bufs=2)
                    nc.vector.tensor_copy(rnm[:], tp[:])
                    nc.sync.dma_start(
                        aginH_own_rag(agin_idx[s], t).bitcast(F32), rnm[:])

            # main columns in two 512-wide sweeps; 2 supports x 4 n-tiles
            # of accumulators fill all 8 PSUM banks per sweep
            for hh in range(2):
                ps_m = {}
                for s in range(2):
                    for n in range(NT):
                        ps_m[(s, n)] = psum.tile(
                            [128, 512], F32, name=f"psm{pid}_{hh}{s}{n}",
                            tag="acc", bufs=8)
                for m in range(MT):
                    mv = sb_mov.tile([128, 512], F32R,
                                     name=f"mv{pid}_{hh}_{m}", tag="mov",
                                     bufs=4)
                    nc.sync.dma_start(mv[:], mov_main(m, hh))
                    for s in range(2):
                        for n in range(NT):
                            nc.tensor.matmul(
                                ps_m[(s, n)][:],
                                T_tile(s, m)[:, n * 128:(n + 1) * 128],
                                mv[:], start=(m == 0), stop=(m == MT - 1))
                for s in range(2):
                    exhs = []
                    for n in range(NT):
                        exh = sb_ex.tile([128, 512], F32,
                                         name=f"ex{pid}{hh}{s}{n}",
                                         tag="ex", bufs=5)
                        nc.vector.tensor_copy(exh[:], ps_m[(s, n)][:])
                        nc.sync.dma_start(
                            aginH_own(agin_idx[s], n, hh).bitcast(F32),
                            exh[:])
                        exhs.append(exh)
                    # feature-major staging: per bc row-block j, transpose
                    # the 4 n-chunks and write one contiguous row-block
                    for j in range(4):
                        st4 = sb_sm.tile([128, NOWN], F32,
                                         name=f"st4{pid}", tag="st", bufs=2)
                        for n in range(NT):
                            tp = psum.tile([128, 128], F32,
                                           name=f"tp{pid}", tag="acc",
                                           bufs=8)
                            nc.tensor.transpose(
                                tp[:], exhs[n][:, j * 128:(j + 1) * 128],
                                ident[:])
                            nc.vector.tensor_copy(
                                st4[:, n * 128:(n + 1) * 128], tp[:])
                        jj = hh * 4 + j
                        nc.sync.dma_start(
                            yt_dst[s].opt()[jj * 128:(jj + 1) * 128, :]
                            .bitcast(F32), st4[:])
                if hh == 0:
                    for s in range(2):
                        nc.gpsimd.collective_compute(
                            "AllGather", mybir.AluOpType.bypass,
                            replica_groups=GROUP,
                            ins=[ag_inA[agin_idx[s]].opt()],
                            outs=[ag_outA[agin_idx[s]].opt()])
            for s in range(2):
                nc.gpsimd.collective_compute(
                    "AllGather", mybir.AluOpType.bypass,
                    replica_groups=GROUP,
                    ins=[ag_inB[agin_idx[s]].opt()],
                    outs=[ag_outB[agin_idx[s]].opt()])


        # ======= hop-2 product: transposed form (feature-major out) =======
        def emit_hop2(pid, s, ag_idx, ownT_rows, yt_dst):
            """Y2^T[bc, own n] = 2*(A_s @ Y1)^T[bc, n] - X^T[bc, n].

            Moving operand = gathered Y1 (ag_out[ag_idx]) loaded as full
            m-rows; its 128-col slices act as lhsT for 8 concurrent
            feature-tile accumulators. ownT_rows(j, w) gives X^T rows for
            the -X term."""
            # ragged feature tile (j = JT), its own accumulation
            ps_r = psum.tile([RAG, NOWN], F32, name=f"ph2r{pid}", tag="acc",
                             bufs=8)
            for m in range(MT):
                mvr = sb_mov.tile([128, RAG], F32R, name=f"mvr{pid}_{m}",
                                  tag="movr", bufs=8)
                nc.sync.dma_start(mvr[:], agH_rag(ag_idx, m))
                nc.tensor.matmul(ps_r[:], mvr[:], T_tile(s, m),
                                 start=(m == 0), stop=False)
            xrt = sb_mov.tile([RAG, NOWN], F32R, name=f"xrt{pid}r",
                              tag="xrt", bufs=2)
            nc.sync.dma_start(xrt[:], ownT_rows(JT, RAG))
            nc.tensor.matmul(ps_r[:], nident[0:RAG, 0:RAG], xrt[:],
                             start=False, stop=True)
            exr = sb_ex.tile([RAG, NOWN], F32, name=f"h2exr{pid}",
                             tag="ex", bufs=5)
            nc.scalar.mul(exr[:], ps_r[:], 2.0)
            nc.sync.dma_start(
                yt_dst.opt()[MAIN:BC, :].bitcast(F32), exr[:])

            # 8 full feature tiles, m-outer (row loads are contiguous)
            ps = [psum.tile([128, NOWN], F32, name=f"ph2{pid}_{j}",
                            tag="acc", bufs=8) for j in range(JT)]
            for m in range(MT):
                mr = [sb_mov.tile([128, 512], F32R,
                                  name=f"mr{pid}_{m}_{h}", tag="mov",
                                  bufs=4) for h in range(2)]
                for h in range(2):
                    nc.sync.dma_start(mr[h][:], agH_main(ag_idx, m, h))
                for j in range(JT):
                    nc.tensor.matmul(
                        ps[j][:],
                        mr[j // 4][:, (j % 4) * 128:(j % 4 + 1) * 128],
                        T_tile(s, m), start=(m == 0), stop=False)
            for j in range(JT):
                xrt = sb_mov.tile([128, NOWN], F32R, name=f"xrt{pid}_{j}",
                                  tag="xrt", bufs=2)
                nc.sync.dma_start(xrt[:], ownT_rows(j, 128))
                nc.tensor.matmul(ps[j][:], nident[:], xrt[:],
                                 start=False, stop=True)
                exh = sb_ex.tile([128, NOWN], F32, name=f"h2ex{pid}_{j}",
                                 tag="ex", bufs=5)
                nc.scalar.mul(exh[:], ps[j][:], 2.0)
                nc.sync.dma_start(
                    yt_dst.opt()[j * 128:(j + 1) * 128, :].bitcast(F32),
                    exh[:])

        # ======================= GCN 1 (gate) =======================
        emit_hop1_pair("g1h1", xs_main_half, xs_rag_tile, (0, 1),
                       (yt[0], yt[2]))

        def xsT_orig_rows(j, w):
            return d["xsT_orig"].ap()[j * 128:j * 128 + w, :]

        emit_hop2("g1s0h2", 0, 0, xsT_orig_rows, yt[1])
        emit_hop2("g1s1h2", 1, 1, xsT_orig_rows, yt[3])

        # gate W-stage + candidate build
        for b in range(B):
            xsT_b = sb_sm.tile([C, NOWN], F32R, name="xsTb", tag="xsTb",
                               bufs=2)
            nc.sync.dma_start(xsT_b[:],
                              d["xsT_own"].ap()[b * C:(b + 1) * C, :])
            blocks = [xsT_b]
            for j in range(4):
                bt = sb_sm.tile([C, NOWN], F32R, name=f"blk{j}",
                                tag=f"blk{j}", bufs=2)
                nc.sync.dma_start(bt[:], yt[j].opt()[b * C:(b + 1) * C, :])
                blocks.append(bt)
            zr_ps = psum.tile([2 * H, NOWN], F32, name="zrps", tag="acc", bufs=8)
            for j in range(5):
                nc.tensor.matmul(zr_ps[:], wg_t[:, j, :], blocks[j][:],
                                 start=(j == 0), stop=(j == 4))
            zr = sb_sm.tile([2 * H, NOWN], F32, name="zr", tag="zr", bufs=2)
            nc.scalar.activation(zr[:], zr_ps[:], AF.Sigmoid, bias=bg_t[:])
            nc.sync.dma_start(rt_dram.opt()[b], zr[H:2 * H, :])
            # candT_b rows are [z*state(64); x(2)] (host permutes W rows)
            cT = sb_sm.tile([C, NOWN], F32, name="cT", tag="cT", bufs=1)
            nc.vector.tensor_mul(cT[0:H, :], zr[0:H, :],
                                 xsT_b[0:H, :].bitcast(F32))
            nc.vector.tensor_copy(cT[H:C, :], xsT_b[H:C, :].bitcast(F32))
            nc.sync.dma_start(
                candT_dram.opt()[b * C:(b + 1) * C, :].bitcast(F32), cT[:])
            # cand node-major -> ag_in[2]
            a_main = ag_in[2].opt()[0:MAIN_ELEMS].rearrange(
                "(p f) -> p f", f=MAIN).bitcast(F32)
            for t in range(NT):
                ps = psum.tile([128, C], F32, name="ctps", tag="acc", bufs=8)
                nc.tensor.transpose(ps[:], cT[:, t * 128:(t + 1) * 128],
                                    ident[0:C, 0:C])
                ct_nm = sb_sm.tile([128, C], F32, name="ctnm", tag="ctnm", bufs=1)
                nc.vector.tensor_copy(ct_nm[:], ps[:])
                lo, hi = b * C, (b + 1) * C
                if hi <= MAIN:
                    nc.sync.dma_start(
                        a_main[t * 128:(t + 1) * 128, lo:hi], ct_nm[:])
                else:
                    cut = MAIN - lo
                    nc.sync.dma_start(
                        a_main[t * 128:(t + 1) * 128, lo:MAIN],
                        ct_nm[:, 0:cut])
                    nc.sync.dma_start(agin_own_rag(2, t).bitcast(F32),
                                      ct_nm[:, cut:C])
        nc.gpsimd.collective_compute(
            "AllGather", mybir.AluOpType.bypass, replica_groups=GROUP,
            ins=[ag_in[2].opt()], outs=[ag_out[2].opt()])

        # ======================= GCN 2 (update) =======================
        emit_hop1_pair("g2h1",
                       lambda m, h: ag_main_half(2, m, h),
                       lambda m: ag_rag_tile(2, m), (3, 4),
                       (yt2[0], yt2[2]))

        def candT_rows(j, w):
            return candT_dram.opt()[j * 128:j * 128 + w, :]

        emit_hop2("g2s0h2", 0, 3, candT_rows, yt2[1])
        emit_hop2("g2s1h2", 1, 4, candT_rows, yt2[3])

        # update W-stage + final combine
        for b in range(B):
            cT_b = sb_sm.tile([C, NOWN], F32R, name="cTb", tag="xsTb",
                              bufs=2)
            nc.sync.dma_start(cT_b[:],
                              candT_dram.opt()[b * C:(b + 1) * C, :])
            blocks = [cT_b]
            for j in range(4):
                bt = sb_sm.tile([C, NOWN], F32R, name=f"ublk{j}",
                                tag=f"blk{j}", bufs=2)
                nc.sync.dma_start(bt[:], yt2[j].opt()[b * C:(b + 1) * C, :])
                blocks.append(bt)
            hc_ps = psum.tile([H, NOWN], F32, name="hcps", tag="acc", bufs=8)
            for j in range(5):
                nc.tensor.matmul(hc_ps[:], wu_t[:, j, :], blocks[j][:],
                                 start=(j == 0), stop=(j == 4))
            hc = sb_sm.tile([H, NOWN], F32, name="hc", tag="zr", bufs=2)
            nc.scalar.activation(hc[:], hc_ps[:], AF.Tanh, bias=bu_t[:])

            # out = hc + r * (state - hc);  stateT = xsT_own rows [0:H]
            xsT_b = sb_sm.tile([C, NOWN], F32, name="xsTb2", tag="cT",
                               bufs=1)
            nc.sync.dma_start(
                xsT_b[:],
                d["xsT_own"].ap()[b * C:(b + 1) * C, :].bitcast(F32))
            rT = sb_sm.tile([H, NOWN], F32, name="rT", tag="rT", bufs=1)
            nc.sync.dma_start(rT[:], rt_dram.opt()[b])
            tmp = sb_sm.tile([H, NOWN], F32, name="tmp", tag="tmp", bufs=2)
            nc.vector.tensor_sub(tmp[:], xsT_b[0:H, :], hc[:])
            nc.vector.tensor_mul(tmp[:], rT[:], tmp[:])
            ot = sb_sm.tile([H, NOWN], F32, name="ot", tag="ot", bufs=2)
            nc.vector.tensor_add(ot[:], hc[:], tmp[:])
            nc.sync.dma_start(d["outT"].ap()[b], ot[:])


def prepare_in_maps(x, state, support0, support1, W_gate, b_gate,
                    W_update, b_update):
    xs = np.concatenate([x, state], axis=-1)          # [B, N, C]
    xs_nm = np.ascontiguousarray(
        xs.transpose(1, 0, 2).reshape(N, BC)).astype(np.float32)
    # feature-major input for W / elementwise uses [state(64); x(2)] rows
    sx_nm = np.ascontiguousarray(
        np.concatenate([state, x], axis=-1)
        .transpose(1, 0, 2).reshape(N, BC)).astype(np.float32)
    perm = np.r_[DIN:C, 0:DIN]                 # [x, state] -> [state, x]
    Wg_dev = np.ascontiguousarray(W_gate, dtype=np.float32).copy()
    Wg_dev[0:C] = Wg_dev[0:C][perm]            # only the X-block reads xsT
    Wu_dev = np.ascontiguousarray(W_update, dtype=np.float32).copy()
    for j in range(5):                         # all of cand's blocks permute
        Wu_dev[j * C:(j + 1) * C] = Wu_dev[j * C:(j + 1) * C][perm]
    xs_main = np.ascontiguousarray(xs_nm[:, :MAIN])
    xs_rag = np.ascontiguousarray(xs_nm[:, MAIN:]).reshape(MT, 128, RAG)
    bg = np.ascontiguousarray(b_gate, dtype=np.float32).reshape(2 * H, 1)
    bu = np.ascontiguousarray(b_update, dtype=np.float32).reshape(H, 1)
    negI = (-0.5 * np.eye(128, dtype=np.float32))

    in_maps = []
    for r in range(NCORES):
        n0 = r * NOWN
        sl = xs_nm[n0:n0 + NOWN]
        in_maps.append({
            "Ts": np.ascontiguousarray(
                np.stack([support0[n0:n0 + NOWN, :].T,
                          support1[n0:n0 + NOWN, :].T])).astype(np.float32),
            "xs_main": xs_main,
            "xs_rag": xs_rag,
            "xsT_orig": np.ascontiguousarray(sl.T),
            "xsT_own": np.ascontiguousarray(sx_nm[n0:n0 + NOWN].T),
            "Wg": Wg_dev, "bg": bg, "Wu": Wu_dev, "bu": bu,
            "negI": negI,
        })
    return in_maps


def assemble_output(results):
    out = np.empty((B, N, H), dtype=np.float32)
    for r in range(NCORES):
        n0 = r * NOWN
        out[:, n0:n0 + NOWN, :] = results[r]["outT"].transpose(0, 2, 1)
    return out


def get_nc():
    if "nc" not in _NC_CACHE:
        _NC_CACHE["nc"] = build_nc()
    return _NC_CACHE["nc"]


def kernel(x, state, support0, support1, W_gate, b_gate, W_update, b_update):
    nc = get_nc()
    in_maps = prepare_in_maps(x, state, support0, support1,
                              W_gate, b_gate, W_update, b_update)
    prev = os.environ.get("BASS_NEVER_TRACE")
    os.environ["BASS_NEVER_TRACE"] = "1"
    try:
        res = run_bass_kernel_spmd(nc, in_maps, list(range(NCORES)),
                                   trace=False)
    finally:
        if prev is None:
            os.environ.pop("BASS_NEVER_TRACE", None)
        else:
            os.environ["BASS_NEVER_TRACE"] = prev
    return assemble_output(res.results)


# revision 27
# speedup vs baseline: 1.0680x; 1.0207x over previous
"""DCGRU cell (nn_DCGRUCell) Trainium2 Bass kernel, 8 NeuronCores.

Sharding: node dimension N=4096 split 8 ways (512 rows/core); supports are
fed host-transposed (T = A^T) so tensor-engine matmuls need no on-device
transposes of A. Hop-1 diffusion products are computed node-major
[node, (batch, feat)] and AllGathered across cores; hop-2 products are
computed directly in transposed (feature-major) form since they only feed
the dense W stage. All matmuls run fp32 (float32r streaming); PSUM fp32.

kernel(**inputs) takes the FULL inputs from reference.setup_inputs() and
returns the FULL [16, 4096, 64] float32 output.
"""
import os
import numpy as np

import concourse.bass as bass
import concourse.mybir as mybir
import concourse.tile as tile
from concourse import bacc
from concourse.bass_utils import run_bass_kernel_spmd

F32 = mybir.dt.float32
F32R = mybir.dt.float32r
AF = mybir.ActivationFunctionType

NCORES = 8
B, N, H, DIN = 16, 4096, 64, 2
C = DIN + H                 # 66 features per batch into each GCN
BC = B * C                  # 1056
NOWN = N // NCORES          # 512 rows per core
NT = NOWN // 128            # 4 n-tiles per core
MT = N // 128               # 32 m-tiles (contraction)
MAIN = 1024                 # bc columns in the node-major main sweep
RAG = BC - MAIN             # 32 ragged columns
JT = BC // 128              # 8 full 128-col feature tiles (+1 ragged)
MAIN_ELEMS = NOWN * MAIN
RAG_ELEMS = NT * 128 * RAG
SHARD = MAIN_ELEMS + RAG_ELEMS
GROUP = [list(range(NCORES))]

_NC_CACHE = {}


def build_nc():
    nc = bacc.Bacc("TRN2", target_bir_lowering=False, debug=False,
                   num_devices=NCORES)

    d = {}
    d["Ts"] = nc.dram_tensor("Ts", [2, N, NOWN], F32R, kind="ExternalInput")
    d["xs_main"] = nc.dram_tensor("xs_main", [N, MAIN], F32R,
                                  kind="ExternalInput")
    d["xs_rag"] = nc.dram_tensor("xs_rag", [MT, 128, RAG], F32R,
                                 kind="ExternalInput")
    d["xsT_orig"] = nc.dram_tensor("xsT_orig", [BC, NOWN], F32R,
                                   kind="ExternalInput")
    d["xsT_own"] = nc.dram_tensor("xsT_own", [BC, NOWN], F32R,
                                  kind="ExternalInput")
    d["Wg"] = nc.dram_tensor("Wg", [5 * C, 2 * H], F32R, kind="ExternalInput")
    d["bg"] = nc.dram_tensor("bg", [2 * H, 1], F32, kind="ExternalInput")
    d["Wu"] = nc.dram_tensor("Wu", [5 * C, H], F32R, kind="ExternalInput")
    d["bu"] = nc.dram_tensor("bu", [H, 1], F32, kind="ExternalInput")
    d["negI"] = nc.dram_tensor("negI", [128, 128], F32R, kind="ExternalInput")
    d["outT"] = nc.dram_tensor("outT", [B, H, NOWN], F32,
                               kind="ExternalOutput")

    with tile.TileContext(nc) as tc:
        _emit(nc, tc, d)
    nc.compile()
    return nc


def _emit(nc, tc, d):
    import contextlib
    stack = contextlib.ExitStack()
    with stack:
        const = stack.enter_context(tc.tile_pool(name="const", bufs=1))
        sb_ex = stack.enter_context(tc.tile_pool(name="ex", bufs=1))
        sb_mov = stack.enter_context(tc.tile_pool(name="mov", bufs=1))
        sb_sm = stack.enter_context(tc.tile_pool(name="small", bufs=1))
        dram = stack.enter_context(
            tc.tile_pool(name="dram", bufs=1, space="DRAM"))
        psum = stack.enter_context(
            tc.tile_pool(name="psum", bufs=1, space="PSUM"))

        # ---- constants / resident tensors ----
        # supports loaded in interleaved 4-m-tile chunks so the first
        # matmuls only wait for the first small chunk
        CH = 4
        NCH = MT // CH
        Tch = {}
        for s in range(2):
            for k in range(NCH):
                Tch[(s, k)] = const.tile([128, CH, 512], F32R,
                                         name=f"T{s}_{k}")
        for k in range(NCH):
            for s in range(2):
                ts = d["Ts"].ap()[s].rearrange("(t p) n -> p t n", p=128)
                nc.sync.dma_start(Tch[(s, k)][:],
                                  ts[:, k * CH:(k + 1) * CH, :])

        def T_tile(s, m):
            return Tch[(s, m // CH)][:, m % CH, :]

        ident = const.tile([128, 128], F32)
        nc.gpsimd.memset(ident[:], 0.0)
        nc.gpsimd.affine_select(
            out=ident[:], in_=ident[:],
            compare_op=mybir.AluOpType.not_equal, fill=1.0, base=0,
            pattern=[[-1, 128]], channel_multiplier=1)
        nident = const.tile([128, 128], F32R)  # -0.5*I (for 2*(Ax - X/2))
        nc.sync.dma_start(nident[:], d["negI"].ap())

        wg_t = const.tile([C, 5, 2 * H], F32R)
        wu_t = const.tile([C, 5, H], F32R)
        for j in range(5):
            nc.sync.dma_start(wg_t[:, j, :],
                              d["Wg"].ap()[j * C:(j + 1) * C, :])
            nc.sync.dma_start(wu_t[:, j, :],
                              d["Wu"].ap()[j * C:(j + 1) * C, :])
        bg_t = const.tile([2 * H, 1], F32)
        nc.sync.dma_start(bg_t[:], d["bg"].ap())
        bu_t = const.tile([H, 1], F32)
        nc.sync.dma_start(bu_t[:], d["bu"].ap())

        # ---- DRAM staging ----
        # AG slots: 0,1 = y1 of gcn1; 2 = cand; 3,4 = y1 of gcn2
        ag_in = [dram.tile([SHARD], F32R, name=f"agin{i}") for i in range(5)]
        ag_out = [dram.tile([NCORES * SHARD], F32R, name=f"agout{i}",
                            addr_space="Shared") for i in range(5)]
        yt = [dram.tile([BC, NOWN], F32R, name=f"yt{i}") for i in range(4)]
        yt2 = [dram.tile([BC, NOWN], F32R, name=f"yt2_{i}") for i in range(4)]
        candT_dram = dram.tile([BC, NOWN], F32R)
        rt_dram = dram.tile([B, H, NOWN], F32)

        def xs_main_half(m, h):
            return d["xs_main"].ap()[m * 128:(m + 1) * 128,
                                     h * 512:(h + 1) * 512]

        def xs_rag_tile(m):
            return d["xs_rag"].ap()[m]

        def ag_main_half(i, m, h):
            a = ag_out[i].opt()
            off = (m // NT) * SHARD + (m % NT) * 128 * MAIN
            v = a[off:off + 128 * MAIN].rearrange("(p f) -> p f", f=MAIN)
            return v[:, h * 512:(h + 1) * 512]

        def ag_rag_tile(i, m):
            a = ag_out[i].opt()
            off = (m // NT) * SHARD + MAIN_ELEMS + (m % NT) * 128 * RAG
            return a[off:off + 128 * RAG].rearrange("(p f) -> p f", f=RAG)

        def ag_col_chunk(i, q, j):
            """[128, NT, 128] moving chunk: rank q's own rows, feature
            columns 128j..128(j+1)."""
            a = ag_out[i].opt()
            blk = a[q * SHARD:q * SHARD + MAIN_ELEMS].rearrange(
                "(t p f) -> p t f", p=128, f=MAIN)
            return blk[:, :, j * 128:(j + 1) * 128]

        def agin_own_main(i, t):
            a = ag_in[i].opt()
            return a[t * 128 * MAIN:(t + 1) * 128 * MAIN].rearrange(
                "(p f) -> p f", f=MAIN)

        def agin_own_main_half(i, t, h):
            return agin_own_main(i, t)[:, h * 512:(h + 1) * 512]

        def agin_own_rag(i, t):
            a = ag_in[i].opt()
            off = MAIN_ELEMS + t * 128 * RAG
            return a[off:off + 128 * RAG].rearrange("(p f) -> p f", f=RAG)

        # ============ hop-1 products: node-major + AllGather ============
        # Both supports share one pass over the moving operand.
        def emit_hop1_pair(pid, mov_main, mov_rag, agin_idx, yt_dst):
            """Y1_s[own rows, :] = A_s @ M for s in (0, 1)."""
            # ragged columns, transposed: psum[c(32), n(512)] per support
            ps_t = [psum.tile([RAG, NOWN], F32, name=f"pst{pid}{s}",
                              tag="acc", bufs=8) for s in range(2)]
            for m in range(MT):
                mvr = sb_mov.tile([128, RAG], F32R, name=f"mvr{pid}_{m}",
                                  tag="movr", bufs=8)
                nc.sync.dma_start(mvr[:], mov_rag(m))
                for s in range(2):
                    nc.tensor.matmul(ps_t[s][:], mvr[:], T_tile(s, m),
                                     start=(m == 0), stop=(m == MT - 1))
            for s in range(2):
                rag_ex = sb_sm.tile([RAG, NOWN], F32, name=f"rgex{pid}{s}",
                                    tag="ragex", bufs=1)
                nc.vector.tensor_copy(rag_ex[:], ps_t[s][:])
                nc.sync.dma_start(yt_dst[s].opt()[MAIN:BC, :].bitcast(F32),
                                  rag_ex[:])
                for t in range(NT):
                    tp = psum.tile([128, RAG], F32, name=f"rtp{pid}{s}",
                                   tag="acc", bufs=8)
                    nc.tensor.transpose(
                        tp[:], rag_ex[:, t * 128:(t + 1) * 128],
                        ident[0:RAG, 0:RAG])
                    rnm = sb_sm.tile([128, RAG], F32, name=f"rnm{pid}{s}",
                                     tag="rnm", bufs=2)
                    nc.vector.tensor_copy(rnm[:], tp[:])
                    nc.sync.dma_start(
                        agin_own_rag(agin_idx[s], t).bitcast(F32), rnm[:])

            # main columns in two 512-wide sweeps; 2 supports x 4 n-tiles
            # of accumulators fill all 8 PSUM banks per sweep
            for hh in range(2):
                ps_m = {}
                for s in range(2):
                    for n in range(NT):
                        ps_m[(s, n)] = psum.tile(
                            [128, 512], F32, name=f"psm{pid}_{hh}{s}{n}",
                            tag="acc", bufs=8)
                for m in range(MT):
                    mv = sb_mov.tile([128, 512], F32R,
                                     name=f"mv{pid}_{hh}_{m}", tag="mov",
                                     bufs=4)
                    nc.sync.dma_start(mv[:], mov_main(m, hh))
                    for s in range(2):
                        for n in range(NT):
                            nc.tensor.matmul(
                                ps_m[(s, n)][:],
                                T_tile(s, m)[:, n * 128:(n + 1) * 128],
                                mv[:], start=(m == 0), stop=(m == MT - 1))
                for s in range(2):
                    exhs = []
                    for n in range(NT):
                        exh = sb_ex.tile([128, 512], F32,
                                         name=f"ex{pid}{hh}{s}{n}",
                                         tag="ex", bufs=6)
                        nc.vector.tensor_copy(exh[:], ps_m[(s, n)][:])
                        nc.sync.dma_start(
                            agin_own_main_half(agin_idx[s], n, hh)
                            .bitcast(F32), exh[:])
                        exhs.append(exh)
                    # feature-major staging: per bc row-block j, transpose
                    # the 4 n-chunks and write one contiguous row-block
                    for j in range(4):
                        st4 = sb_sm.tile([128, NOWN], F32,
                                         name=f"st4{pid}", tag="st", bufs=2)
                        for n in range(NT):
                            tp = psum.tile([128, 128], F32,
                                           name=f"tp{pid}", tag="acc",
                                           bufs=8)
                            nc.tensor.transpose(
                                tp[:], exhs[n][:, j * 128:(j + 1) * 128],
                                ident[:])
                            nc.vector.tensor_copy(
                                st4[:, n * 128:(n + 1) * 128], tp[:])
                        jj = hh * 4 + j
                        nc.sync.dma_start(
                            yt_dst[s].opt()[jj * 128:(jj + 1) * 128, :]
                            .bitcast(F32), st4[:])
            nc.gpsimd.collective_compute(
                "AllGather", mybir.AluOpType.bypass, replica_groups=GROUP,
                ins=[ag_in[agin_idx[0]].opt()],
                outs=[ag_out[agin_idx[0]].opt()])
            nc.gpsimd.collective_compute(
                "AllGather", mybir.AluOpType.bypass, replica_groups=GROUP,
                ins=[ag_in[agin_idx[1]].opt()],
                outs=[ag_out[agin_idx[1]].opt()])

        # ======= hop-2 product: transposed form (feature-major out) =======
        def emit_hop2(pid, s, ag_idx, ownT_rows, yt_dst):
            """Y2^T[bc, own n] = 2*(A_s @ Y1)^T[bc, n] - X^T[bc, n].

            Moving operand = gathered Y1 (ag_out[ag_idx]) loaded as full
            m-rows; its 128-col slices act as lhsT for 8 concurrent
            feature-tile accumulators. ownT_rows(j, w) gives X^T rows for
            the -X term."""
            # ragged feature tile (j = JT), its own accumulation
            ps_r = psum.tile([RAG, NOWN], F32, name=f"ph2r{pid}", tag="acc",
                             bufs=8)
            for m in range(MT):
                mvr = sb_mov.tile([128, RAG], F32R, name=f"mvr{pid}_{m}",
                                  tag="movr", bufs=8)
                nc.sync.dma_start(mvr[:], ag_rag_tile(ag_idx, m))
                nc.tensor.matmul(ps_r[:], mvr[:], T_tile(s, m),
                                 start=(m == 0), stop=False)
            xrt = sb_mov.tile([RAG, NOWN], F32R, name=f"xrt{pid}r",
                              tag="xrt", bufs=2)
            nc.sync.dma_start(xrt[:], ownT_rows(JT, RAG))
            nc.tensor.matmul(ps_r[:], nident[0:RAG, 0:RAG], xrt[:],
                             start=False, stop=True)
            exr = sb_ex.tile([RAG, NOWN], F32, name=f"h2exr{pid}",
                             tag="ex", bufs=6)
            nc.scalar.mul(exr[:], ps_r[:], 2.0)
            nc.sync.dma_start(
                yt_dst.opt()[MAIN:BC, :].bitcast(F32), exr[:])

            # 8 full feature tiles, m-outer (row loads are contiguous)
            ps = [psum.tile([128, NOWN], F32, name=f"ph2{pid}_{j}",
                            tag="acc", bufs=8) for j in range(JT)]
            for m in range(MT):
                mrow = sb_mov.tile([128, MAIN], F32R, name=f"mr{pid}_{m}",
                                   tag="mov", bufs=4)
                for h in range(2):
                    nc.sync.dma_start(mrow[:, h * 512:(h + 1) * 512],
                                      ag_main_half(ag_idx, m, h))
                for j in range(JT):
                    nc.tensor.matmul(
                        ps[j][:], mrow[:, j * 128:(j + 1) * 128],
                        T_tile(s, m), start=(m == 0), stop=False)
            for j in range(JT):
                xrt = sb_mov.tile([128, NOWN], F32R, name=f"xrt{pid}_{j}",
                                  tag="xrt", bufs=2)
                nc.sync.dma_start(xrt[:], ownT_rows(j, 128))
                nc.tensor.matmul(ps[j][:], nident[:], xrt[:],
                                 start=False, stop=True)
                exh = sb_ex.tile([128, NOWN], F32, name=f"h2ex{pid}_{j}",
                                 tag="ex", bufs=6)
                nc.scalar.mul(exh[:], ps[j][:], 2.0)
                nc.sync.dma_start(
                    yt_dst.opt()[j * 128:(j + 1) * 128, :].bitcast(F32),
                    exh[:])

        # ======================= GCN 1 (gate) =======================
        emit_hop1_pair("g1h1", xs_main_half, xs_rag_tile, (0, 1),
                       (yt[0], yt[2]))

        def xsT_orig_rows(j, w):
            return d["xsT_orig"].ap()[j * 128:j * 128 + w, :]

        emit_hop2("g1s0h2", 0, 0, xsT_orig_rows, yt[1])
        emit_hop2("g1s1h2", 1, 1, xsT_orig_rows, yt[3])

        # gate W-stage + candidate build
        for b in range(B):
            xsT_b = sb_sm.tile([C, NOWN], F32R, name="xsTb", tag="xsTb",
                               bufs=2)
            nc.sync.dma_start(xsT_b[:],
                              d["xsT_own"].ap()[b * C:(b + 1) * C, :])
            blocks = [xsT_b]
            for j in range(4):
                bt = sb_sm.tile([C, NOWN], F32R, name=f"blk{j}",
                                tag=f"blk{j}", bufs=2)
                nc.sync.dma_start(bt[:], yt[j].opt()[b * C:(b + 1) * C, :])
                blocks.append(bt)
            zr_ps = psum.tile([2 * H, NOWN], F32, name="zrps", tag="acc", bufs=8)
            for j in range(5):
                nc.tensor.matmul(zr_ps[:], wg_t[:, j, :], blocks[j][:],
                                 start=(j == 0), stop=(j == 4))
            zr = sb_sm.tile([2 * H, NOWN], F32, name="zr", tag="zr", bufs=1)
            nc.scalar.activation(zr[:], zr_ps[:], AF.Sigmoid, bias=bg_t[:])
            nc.sync.dma_start(rt_dram.opt()[b], zr[H:2 * H, :])
            # candT_b rows are [z*state(64); x(2)] (host permutes W rows)
            cT = sb_sm.tile([C, NOWN], F32, name="cT", tag="cT", bufs=1)
            nc.vector.tensor_mul(cT[0:H, :], zr[0:H, :],
                                 xsT_b[0:H, :].bitcast(F32))
            nc.vector.tensor_copy(cT[H:C, :], xsT_b[H:C, :].bitcast(F32))
            nc.sync.dma_start(
                candT_dram.opt()[b * C:(b + 1) * C, :].bitcast(F32), cT[:])
            # cand node-major -> ag_in[2]
            a_main = ag_in[2].opt()[0:MAIN_ELEMS].rearrange(
                "(p f) -> p f", f=MAIN).bitcast(F32)
            for t in range(NT):
                ps = psum.tile([128, C], F32, name="ctps", tag="acc", bufs=8)
                nc.tensor.transpose(ps[:], cT[:, t * 128:(t + 1) * 128],
                                    ident[0:C, 0:C])
                ct_nm = sb_sm.tile([128, C], F32, name="ctnm", tag="ctnm", bufs=1)
                nc.vector.tensor_copy(ct_nm[:], ps[:])
                lo, hi = b * C, (b + 1) * C
                if hi <= MAIN:
                    nc.sync.dma_start(
                        a_main[t * 128:(t + 1) * 128, lo:hi], ct_nm[:])
                else:
                    cut = MAIN - lo
                    nc.sync.dma_start(
                        a_main[t * 128:(t + 1) * 128, lo:MAIN],
                        ct_nm[:, 0:cut])
                    nc.sync.dma_start(agin_own_rag(2, t).bitcast(F32),
                                      ct_nm[:, cut:C])
        nc.gpsimd.collective_compute(
            "AllGather", mybir.AluOpType.bypass, replica_groups=GROUP,
            ins=[ag_in[2].opt()], outs=[ag_out[2].opt()])

        # ======================= GCN 2 (update) =======================
        emit_hop1_pair("g2h1",
                       lambda m, h: ag_main_half(2, m, h),
                       lambda m: ag_rag_tile(2, m), (3, 4),
                       (yt2[0], yt2[2]))

        def candT_rows(j, w):
            return candT_dram.opt()[j * 128:j * 128 + w, :]

        emit_hop2("g2s0h2", 0, 3, candT_rows, yt2[1])
        emit_hop2("g2s1h2", 1, 4, candT_rows, yt2[3])

        # update W-stage + final combine
        for b in range(B):
            cT_b = sb_sm.tile([C, NOWN], F32R, name="cTb", tag="xsTb",
                              bufs=2)
            nc.sync.dma_start(cT_b[:],
                              candT_dram.opt()[b * C:(b + 1) * C, :])
            blocks = [cT_b]
            for j in range(4):
                bt = sb_sm.tile([C, NOWN], F32R, name=f"ublk{j}",
                                tag=f"blk{j}", bufs=2)
                nc.sync.dma_start(bt[:], yt2[j].opt()[b * C:(b + 1) * C, :])
                blocks.append(bt)
            hc_ps = psum.tile([H, NOWN], F32, name="hcps", tag="acc", bufs=8)
            for j in range(5):
                nc.tensor.matmul(hc_ps[:], wu_t[:, j, :], blocks[j][:],
                                 start=(j == 0), stop=(j == 4))
            hc = sb_sm.tile([H, NOWN], F32, name="hc", tag="zr", bufs=1)
            nc.scalar.activation(hc[:], hc_ps[:], AF.Tanh, bias=bu_t[:])

            # out = hc + r * (state - hc);  stateT = xsT_own rows [0:H]
            xsT_b = sb_sm.tile([C, NOWN], F32, name="xsTb2", tag="cT",
                               bufs=1)
            nc.sync.dma_start(
                xsT_b[:],
                d["xsT_own"].ap()[b * C:(b + 1) * C, :].bitcast(F32))
            rT = sb_sm.tile([H, NOWN], F32, name="rT", tag="rT", bufs=1)
            nc.sync.dma_start(rT[:], rt_dram.opt()[b])
            tmp = sb_sm.tile([H, NOWN], F32, name="tmp", tag="tmp", bufs=2)
            nc.vector.tensor_sub(tmp[:], xsT_b[0:H, :], hc[:])
            nc.vector.tensor_mul(tmp[:], rT[:], tmp[:])
            ot = sb_sm.tile([H, NOWN], F32, name="ot", tag="ot", bufs=2)
            nc.vector.tensor_add(ot[:], hc[:], tmp[:])
            nc.sync.dma_start(d["outT"].ap()[b], ot[:])


def prepare_in_maps(x, state, support0, support1, W_gate, b_gate,
                    W_update, b_update):
    xs = np.concatenate([x, state], axis=-1)          # [B, N, C]
    xs_nm = np.ascontiguousarray(
        xs.transpose(1, 0, 2).reshape(N, BC)).astype(np.float32)
    # feature-major input for W / elementwise uses [state(64); x(2)] rows
    sx_nm = np.ascontiguousarray(
        np.concatenate([state, x], axis=-1)
        .transpose(1, 0, 2).reshape(N, BC)).astype(np.float32)
    perm = np.r_[DIN:C, 0:DIN]                 # [x, state] -> [state, x]
    Wg_dev = np.ascontiguousarray(W_gate, dtype=np.float32).copy()
    Wg_dev[0:C] = Wg_dev[0:C][perm]            # only the X-block reads xsT
    Wu_dev = np.ascontiguousarray(W_update, dtype=np.float32).copy()
    for j in range(5):                         # all of cand's blocks permute
        Wu_dev[j * C:(j + 1) * C] = Wu_dev[j * C:(j + 1) * C][perm]
    xs_main = np.ascontiguousarray(xs_nm[:, :MAIN])
    xs_rag = np.ascontiguousarray(xs_nm[:, MAIN:]).reshape(MT, 128, RAG)
    bg = np.ascontiguousarray(b_gate, dtype=np.float32).reshape(2 * H, 1)
    bu = np.ascontiguousarray(b_update, dtype=np.float32).reshape(H, 1)
    negI = (-0.5 * np.eye(128, dtype=np.float32))

    in_maps = []
    for r in range(NCORES):
        n0 = r * NOWN
        sl = xs_nm[n0:n0 + NOWN]
        in_maps.append({
            "Ts": np.ascontiguousarray(
                np.stack([support0[n0:n0 + NOWN, :].T,
                          support1[n0:n0 + NOWN, :].T])).astype(np.float32),
            "xs_main": xs_main,
            "xs_rag": xs_rag,
            "xsT_orig": np.ascontiguousarray(sl.T),
            "xsT_own": np.ascontiguousarray(sx_nm[n0:n0 + NOWN].T),
            "Wg": Wg_dev, "bg": bg, "Wu": Wu_dev, "bu": bu,
            "negI": negI,
        })
    return in_maps


def assemble_output(results):
    out = np.empty((B, N, H), dtype=np.float32)
    for r in range(NCORES):
        n0 = r * NOWN
        out[:, n0:n0 + NOWN, :] = results[r]["outT"].transpose(0, 2, 1)
    return out


def get_nc():
    if "nc" not in _NC_CACHE:
        _NC_CACHE["nc"] = build_nc()
    return _NC_CACHE["nc"]


def kernel(x, state, support0, support1, W_gate, b_gate, W_update, b_update):
    nc = get_nc()
    in_maps = prepare_in_maps(x, state, support0, support1,
                              W_gate, b_gate, W_update, b_update)
    prev = os.environ.get("BASS_NEVER_TRACE")
    os.environ["BASS_NEVER_TRACE"] = "1"
    try:
        res = run_bass_kernel_spmd(nc, in_maps, list(range(NCORES)),
                                   trace=False)
    finally:
        if prev is None:
            os.environ.pop("BASS_NEVER_TRACE", None)
        else:
            os.environ["BASS_NEVER_TRACE"] = prev
    return assemble_output(res.results)
